# revision 39
# baseline (speedup 1.0000x reference)
# kernel.py — ConcatAttention on 8 Trainium2 NeuronCores (Bass/Tile, SPMD, no collectives).
#
# reference math (B=4, S=512, H=512, A=128):
#   a[b,i,:] = lstm[b,i] @ W1^T + W_b          (W1 = W_w[:, :H])
#   c[b,j,:] = lstm[b,j] @ W2^T                (W2 = W_w[:, H:])
#   scores[b,i] = sum_j sum_a tanh(a[b,i,a] + c[b,j,a]) * v[a]
#   attn = softmax(where(i < len_b, scores, -1e9), axis=i)
#   context[b] = sum_i attn[b,i] * lstm[b,i]
#
# Algorithm: per (b, a) the function f(t) = sum_j tanh(t + c[b,j,a]) is analytic on
# t in [-2.56, 2.56] (the range a occupies), so a degree-4 Chebyshev interpolant
# (K=5 nodes) reproduces it to ~9.4e-3 end-to-end relative error (tolerance 2e-2):
#   nodes:  F[a,k] = sum_j tanh(t_k + c[a,j])   -> K fused ACT tanh+accum instrs
#   coeffs: coef = F @ Cmat^T                   -> PE transpose + tiny matmul (DCT)
#   eval:   scores[i] = sum_m (v*coef)[a,m] T_m(tau[a,i]) -> K-1 accumulated PE
#           matmuls over the DVE-built Chebyshev basis.
#
# Perf notes (cost-model driven):
#  - All PE matmul inputs are fp16 (1 cycle/row vs fp32's 4); inputs stream in as
#    fp16, halving DMA bytes. End-to-end precision validated at ~9.4e-3.
#  - The i-mask is applied by one extra accumulated matmul adding a -60000 row
#    (fp16-exact, no infs) into the score PSUM; softmax max/merge handles the rest.
#  - Context is produced in partition layout ([h,4] via N=1 matmuls); the softmax
#    row (e | -m | z) leaves early in its own DMA while context computes.
#  - DMA issues are spread over the HWDGE-capable sequencers (SP/ACT); dummy
#    matmuls keep the PE p-state ramped across its idle windows.
#
# Sharding: core = (batch b = core//2, i-half = core%2). Inputs are rotated on the
# host so every core runs the identical program on "its" first 256 rows; the j-sum
# is permutation invariant. Softmax is flash-style per half (m, z, unnormalized e
# and context); halves merge on the host with two scalars per batch.
#
# walrus codegen allows a single sync-wait per TPB instruction, so per engine a
# cheap "gate" op touches each DMA-fed operand first; every real instruction then
# carries at most one unobserved cross-engine producer.

import numpy as np

import concourse.bass as bass
import concourse.mybir as mybir
import concourse.tile as tile
from concourse import bacc
from concourse.bass_utils import run_bass_kernel_spmd
from concourse.tile_rust import add_dep_helper

F32 = mybir.dt.float32
F16 = mybir.dt.float16
AF = mybir.ActivationFunctionType
OP = mybir.AluOpType

B, S, H, A = 4, 512, 512, 128
SH = S // 2          # 256: per-core i-half
K = 5                # Chebyshev nodes (degree 4)
HALF = 2.56          # tau = a / HALF maps a-range into [-1, 1]
N_CORES = 8
NDVE = 0             # leading nodes whose j-sum runs on DVE instead of ACT
NEGF16 = -60000.0    # fp16-exact "minus infinity" for masked queries
WU = 4               # PE p-state warmup matmuls (front)
WK = 46              # PE keep-warm matmuls through the node phase
CONSTS_SP = False    # consts via SP 3rd HWDGE (False: Pool SWDGE)
C_FIRST = True       # all four c-chunks before the a-chunks
GCON_LATE = False    # PE consts gate after projections

# consts layout (one [128, CW] f32 tensor). fp16 blocks are bitcast f32 columns.
C_IDH = 0              # [:, 0:64]    fp16 identity 128x128 (64 f32 cols)
C_TK = 64              # [:, 64:72]   chebyshev node biases (tiled rows)
C_WBH = 72             # [:, 72:73]   W_b / HALF column
C_VW = 73              # [:, 73:74]   v_w column
C_CM = 74              # [:, 74:82]   DCT matrix tiled: rows 32r+k = Cmat^T[k]
C_NM = 82              # [0:1, 82:210] fp16 mask row: 0 active / -60000 masked
C_ONE = 210            # [0:1, 210:211] fp16 [1.0, 0.0] pair
C_ONEF = 211           # [0:1, 211:212] f32 1.0 (transpose identity scalar)
C_CMT = 212            # [:, 212:261] cmt row k tiled on all partitions, per k
CW = 212 + 49
PD = 192               # Pool pre-delay cols before the consts SWDGE issue


def _build_nc():
    nc = bacc.Bacc("TRN2", target_bir_lowering=False, debug=False,
                   num_devices=N_CORES)

    con_d = nc.dram_tensor("consts", [128, CW], F32, kind="ExternalInput")
    # per-h-chunk interleave: cols 0:256 = wts chunk, 256:768 = xt chunk
    ins_d = nc.dram_tensor("ins", [128, 4, 2 * A + S], F16,
                           kind="ExternalInput")

    # outputs: ctx in partition layout; softmax row (e | -m | z) separately
    out_d = nc.dram_tensor("out_all", [128, 4], F32, kind="ExternalOutput")
    outr_d = nc.dram_tensor("out_row", [1, SH + 2], F32, kind="ExternalOutput")

    with tile.TileContext(nc) as tc:
        with (
            tc.tile_pool(name="sb", bufs=1) as sb,
            tc.tile_pool(name="pc", bufs=1, space=bass.MemorySpace.PSUM) as pc,
            tc.tile_pool(name="pa", bufs=1, space=bass.MemorySpace.PSUM) as pa,
            tc.tile_pool(name="px", bufs=1, space=bass.MemorySpace.PSUM) as px,
            tc.tile_pool(name="pt", bufs=1, space=bass.MemorySpace.PSUM) as pt,
            tc.tile_pool(name="pscr", bufs=2,
                         space=bass.MemorySpace.PSUM) as pscr,
        ):
            # --- input DMAs: one per h-chunk so each c-projection starts as
            # soon as its own chunk lands (SP/ACT alternate HWDGE issues) ----
            ins = sb.tile([128, 4, 2 * A + S], F16)
            nc.sync.dma_start(ins[:, 0, :], ins_d.ap()[:, 0, :])
            nc.scalar.dma_start(ins[:, 1, :], ins_d.ap()[:, 1, :])
            nc.sync.dma_start(ins[:, 2, :], ins_d.ap()[:, 2, :])
            nc.scalar.dma_start(ins[:, 3, :], ins_d.ap()[:, 3, :])

            def wts(hc):
                return ins[:, hc, 0:2 * A]

            def xts(hc, lo, hi):
                return ins[:, hc, 2 * A + lo:2 * A + hi]

            con = sb.tile([128, CW], F32)
            if CONSTS_SP:
                nc.sync.dma_start(con[:, :], con_d.ap())
            else:
                # dummy Pool op first: delays the SWDGE generation just enough
                # that the xt transfers win the shared DMA queue
                pdum = sb.tile([128, PD // 4], F32)
                nc.gpsimd.memset(pdum[:, :], 0.0)
                nc.gpsimd.dma_start(con[:, :], con_d.ap())

            identh = con[:, C_IDH:C_IDH + 64].bitcast(F16)   # [128,128] fp16
            tks = con[:, C_TK:C_TK + K]
            wbh = con[:, C_WBH:C_WBH + 1]
            vw = con[:, C_VW:C_VW + 1]
            cmt4 = con[:, C_CM:C_CM + K]                     # tiled per 32-blk
            nmk = con[0:1, C_NM:C_NM + 128].bitcast(F16)     # [1,256] fp16
            onec = con[0:1, C_ONE:C_ONE + 1].bitcast(F16)    # [1,2] fp16
            onef = con[0:1, C_ONEF:C_ONEF + 1]               # [1,1] f32 1.0
            cmtt = con[:, C_CMT:C_CMT + K * K]                # row-tiled cmt

            # --- engine gates: pre-observe each DMA per engine --------------
            g_wts = nc.tensor.ldweights(ins[:, 0, 0:1])
            if not GCON_LATE:
                g_con = nc.tensor.ldweights(identh[:, 0:1])
                add_dep_helper(g_con.ins, g_wts.ins, False, "gate order")
            dummy_a = sb.tile([A, 1], F32)
            # also preloads the tanh/exp ACT table while DMAs stream
            g_act = nc.scalar.activation(dummy_a[:, :], tks[:, 0:1], AF.Tanh,
                                         bias=tks[:, 0:1])
            dummy_d = sb.tile([1, 1], F32)
            g_dve = nc.vector.tensor_copy(dummy_d[0:1, 0:1], con[0:1, 0:1])

            # --- PE p-state warmup on wts while xt streams ------------------
            # (shares the ftp-tag PSUM bank; warmup is long dead before the
            # DCT transpose reuses it)
            wu_ps = pt.tile([32, 128], F32, tag="ftp")
            for i in range(WU):
                nc.tensor.matmul(wu_ps[0:1, 0:128], ins[:, 0, 0:1],
                                  ins[:, 0, 0:128], start=True, stop=True)

            # --- projections on PE: c first (feeds nodes), then a -----------
            c_ps = pc.tile([A, S], F32)
            a_ps = pa.tile([A, SH], F32, tag="a_ps")
            if C_FIRST:
                order = [("c", 0), ("c", 1), ("c", 2), ("c", 3),
                         ("a", 0), ("a", 1), ("a", 2), ("a", 3)]
            else:
                order = [("c", 0), ("c", 1), ("a", 0), ("a", 1),
                         ("c", 2), ("c", 3), ("a", 2), ("a", 3)]
            for kind, hc in order:
                if kind == "c":
                    nc.tensor.matmul(c_ps[:, :], wts(hc)[:, A:2 * A],
                                     xts(hc, 0, S), start=(hc == 0),
                                     stop=(hc == 3), skip_group_check=True)
                else:
                    nc.tensor.matmul(a_ps[:, :], wts(hc)[:, 0:A],
                                     xts(hc, 0, SH), start=(hc == 0),
                                     stop=(hc == 3), skip_group_check=True)
            if GCON_LATE:
                # PE observes the consts DMA before its first use (transposes)
                g_con = nc.tensor.ldweights(identh[:, 0:1])

            # --- rebuild x[i,h] (fp16) for the context matmul ---------------
            xh0 = sb.tile([128, H], F16)
            xh1 = sb.tile([128, H], F16)
            xh_sb = [xh0, xh1]
            for sc in range(2):
                xps = px.tile([128, 4, 128], F16, tag="xps")
                for hc in range(4):
                    nc.tensor.transpose(xps[:, hc, :],
                                        xts(hc, sc * 128, (sc + 1) * 128),
                                        identh)
                nc.vector.tensor_copy(xh_sb[sc][:, :], xps[:, :, :])

            # --- tau (=T_1) and basis recurrence, all on DVE ----------------
            basis = sb.tile([A, K, SH], F16)  # slots m=1..K-1 used
            nc.vector.tensor_scalar(basis[:, 1, :], a_ps[:, :], 1.0 / HALF,
                                    wbh, OP.mult, OP.add)

            # --- Chebyshev node sums on ACT (tanh + fused row-sum) ----------
            # fnode is padded to 32 cols for the DVE block-transpose; the pad
            # is zeroed on ACT (no cross-engine dep) right after the gate
            fnode = sb.tile([A, 8], F32)
            for k in range(K):
                scr = pscr.tile([A, S], F32, tag="scr")
                nc.scalar.activation(scr[:, :], c_ps[:, :], AF.Tanh,
                                     bias=tks[:, k:k + 1],
                                     accum_out=fnode[:, k:k + 1])

            # --- basis: T_2k = 2*T_k^2-1, T_2k+1 = 2*T_k*T_k+1 - T_1 --------
            um = sb.tile([A, SH], F16)
            for m in range(2, K):
                hm = m // 2 if m % 2 == 0 else (m - 1) // 2
                nc.vector.tensor_mul(um[:, :], basis[:, hm, :],
                                     basis[:, hm + (m % 2), :])
                if m % 2 == 0:
                    nc.vector.tensor_scalar(basis[:, m, :], um[:, :], 2.0,
                                            -1.0, OP.mult, OP.add)
                else:
                    nc.vector.scalar_tensor_tensor(basis[:, m, :], um[:, :],
                                                   2.0, basis[:, 1, :],
                                                   OP.mult, OP.subtract)

            # --- keep PE p-state ramped through the ACT node phase ----------
            for i in range(WK):
                nc.tensor.matmul(wu_ps[0:1, 0:128], ins[:, 0, 0:1],
                                 ins[:, 0, 0:128], start=True, stop=True)

            # --- node values -> v * Chebyshev coefficients ------------------
            # incremental DCT on DVE: after node k lands, rank-1 update
            # coef += F[:,k] (x) cmt[k,:] via a per-partition-scalar op, so
            # only the last tiny update sits after the final node
            tailp = pt.tile([128, 16], F32, tag="tail")
            coefa = sb.tile([A, 8], F32)
            for k in range(K):
                blk = cmtt[:, k * K:(k + 1) * K]
                if k == 0:
                    nc.vector.tensor_scalar(coefa[:, 0:K], blk,
                                            fnode[:, 0:1], None, OP.mult)
                else:
                    nc.vector.scalar_tensor_tensor(coefa[:, 0:K], blk,
                                                   fnode[:, k:k + 1],
                                                   coefa[:, 0:K],
                                                   OP.mult, OP.add)
            vcoef = sb.tile([A, 8], F16)
            nc.vector.tensor_scalar(vcoef[:, 0:K], coefa[:, 0:K], vw, None,
                                    OP.mult)

            # --- scores via K-1 accumulated matmuls + mask row --------------
            sco = pa.tile([1, SH], F32, tag="sco")
            for m in range(1, K):
                nc.tensor.matmul(sco[:, :], vcoef[:, m:m + 1], basis[:, m, :],
                                 start=(m == 1), stop=False,
                                 skip_group_check=True)
            nc.tensor.matmul(sco[:, :], onec[:, 0:1], nmk,
                             start=False, stop=True, skip_group_check=True)

            # --- flash softmax half: negm, e, z -----------------------------
            negm = sb.tile([1, 1], F32)
            nc.vector.tensor_reduce(negm[:, :], sco[:, :],
                                    axis=mybir.AxisListType.X, op=OP.max,
                                    negate=True)
            # ACT observes sco while DVE reduces, so exp carries only negm
            dummy_s = sb.tile([1, 1], F32)
            g_sco = nc.scalar.activation(dummy_s[:, :], sco[0:1, 0:1],
                                         AF.Identity)
            # exp writes straight into the row-packed output tile
            packr = sb.tile([1, SH + 2], F32)
            e_sb = packr[0:1, 0:SH]
            eop = nc.scalar.activation(e_sb, sco[:, :], AF.Exp,
                                       bias=negm[0:1, 0:1])
            add_dep_helper(eop.ins, g_sco.ins, False, "gate order")
            # z on DVE (parallel with the PE e-transposes); negm via ACT after
            zop = nc.vector.tensor_reduce(packr[0:1, SH + 1:SH + 2], e_sb,
                                          axis=mybir.AxisListType.X, op=OP.add)
            pmop = nc.scalar.activation(packr[0:1, SH:SH + 1], negm[:, :],
                                        AF.Identity)
            add_dep_helper(pmop.ins, eop.ins, False, "pack order")
            add_dep_helper(pmop.ins, zop.ins, False, "row pack complete")
            nc.scalar.dma_start(outr_d.ap(), packr[:, :])

            # --- context in partition layout: cux[h,hc] ---------------------
            etp = tailp[:, 8:10]
            for ch in range(2):
                nc.tensor.transpose(etp[:, ch:ch + 1],
                                    e_sb[0:1, ch * 128:(ch + 1) * 128],
                                    onef)
            et = sb.tile([128, 2], F16)
            nc.vector.tensor_copy(et[:, :], etp[:, :])
            cux = tailp[:, 10:14]
            for hc in range(4):
                for sc in range(2):
                    nc.tensor.matmul(cux[:, hc:hc + 1],
                                     xh_sb[sc][:, hc * 128:(hc + 1) * 128],
                                     et[:, sc:sc + 1],
                                     start=(sc == 0), stop=(sc == 1),
                                     skip_group_check=True)

            # --- ctx leaves in a second small DMA ---------------------------
            pack = sb.tile([128, 4], F32)
            nc.vector.tensor_copy(pack[:, :], cux)
            nc.sync.dma_start(out_d.ap(), pack[:, :])

    nc.compile()
    return nc


_NC_CACHE = None


def _get_nc():
    global _NC_CACHE
    if _NC_CACHE is None:
        _NC_CACHE = _build_nc()
    return _NC_CACHE


def _host_inputs(lstm_out, lengths, W_w, W_b, v_w):
    lstm = np.ascontiguousarray(np.asarray(lstm_out), dtype=np.float32)
    W_w = np.asarray(W_w, dtype=np.float32)
    W_b = np.asarray(W_b, dtype=np.float32)
    v_w = np.asarray(v_w, dtype=np.float32)
    lengths = np.asarray(lengths).astype(np.int64)

    wts = np.empty((H, 2 * A), np.float16)
    wts[:, 0:A] = W_w[:, :H].T          # W1^T
    wts[:, A:2 * A] = W_w[:, H:].T      # W2^T
    wts = wts.reshape(4, 128, 2 * A).transpose(1, 0, 2)  # [128, 4hc, 2A]

    kk = np.arange(K)
    tk = (HALF * np.cos((2 * kk + 1) * np.pi / (2 * K))).astype(np.float32)
    mm = np.arange(K)
    cmat = np.cos(np.outer(mm, (2 * kk + 1)) * np.pi / (2 * K)) * (2.0 / K)
    cmat[0] *= 0.5

    con_base = np.zeros((128, CW), np.float32)
    con_base[:, C_IDH:C_IDH + 64] = np.eye(128, dtype=np.float16).view(np.float32)
    con_base[:, C_TK:C_TK + K] = np.tile(tk[None, :], (128, 1))
    con_base[:, C_WBH:C_WBH + 1] = (W_b / np.float32(HALF))[:, None]
    con_base[:, C_VW:C_VW + 1] = v_w[:, None]
    for r in range(4):
        con_base[32 * r:32 * r + K, C_CM:C_CM + K] = cmat.T.astype(np.float32)
    for k in range(K):
        con_base[:, C_CMT + k * K:C_CMT + (k + 1) * K] = \
            cmat[:, k].astype(np.float32)[None, :]
    con_base[0:1, C_ONE:C_ONE + 1] = (
        np.array([[1.0, 0.0]], np.float16).view(np.float32))
    con_base[0, C_ONEF] = 1.0

    mask01 = (np.arange(S)[None, :] < lengths[:, None])

    in_maps = []
    for core in range(N_CORES):
        b, half = core // 2, core % 2
        rot = half * SH
        x_rot = np.concatenate([lstm[b, rot:], lstm[b, :rot]], axis=0)
        x16 = x_rot.astype(np.float16)
        xt = x16.T.reshape(4, 128, S).transpose(1, 0, 2)   # [128, 4, S]
        ins = np.empty((128, 4, 2 * A + S), np.float16)
        ins[:, :, 0:2 * A] = wts
        ins[:, :, 2 * A:] = xt
        m01 = mask01[b, rot:rot + SH]
        con = con_base.copy()
        nmk16 = np.where(m01, np.float16(0.0),
                         np.float16(NEGF16)).astype(np.float16)
        con[0:1, C_NM:C_NM + 128] = nmk16.reshape(1, SH).view(np.float32)
        in_maps.append({
            "consts": con,
            "ins": ins,
        })
    return in_maps


def _combine(results, lengths, lstm):
    attn = np.zeros((B, S), np.float32)
    ctx = np.zeros((B, H), np.float32)
    for b in range(B):
        if lengths[b] == 0:
            attn[b] = 1.0 / S
            ctx[b] = lstm[b].mean(axis=0)
            continue
        mzs, es, cxs = [], [], []
        for h in range(2):
            o = results[2 * b + h]["out_all"].astype(np.float64)
            r = results[2 * b + h]["out_row"][0].astype(np.float64)
            m_h = -r[SH]
            z_h = r[SH + 1]
            e_h = r[0:SH]
            c_h = o[:, 0:4].T.reshape(H)
            mzs.append((m_h, z_h)); es.append(e_h); cxs.append(c_h)
        mg = max(m for m, _ in mzs)
        acc_z = 0.0
        aa = []
        for (m_h, z_h) in mzs:
            a_h = np.exp(m_h - mg) if np.isfinite(m_h) else 0.0
            aa.append(a_h)
            acc_z += a_h * z_h
        attn[b, :SH] = aa[0] * es[0] / acc_z
        attn[b, SH:] = aa[1] * es[1] / acc_z
        ctx[b] = (aa[0] * cxs[0] + aa[1] * cxs[1]) / acc_z
    return ctx, attn


def run(inputs, trace=False):
    """Internal entry that also exposes tracing; returns ((ctx, attn), results)."""
    nc = _get_nc()
    in_maps = _host_inputs(**inputs)
    res = run_bass_kernel_spmd(nc, in_maps, core_ids=list(range(N_CORES)),
                               trace=trace)
    lengths = np.asarray(inputs["lengths"]).astype(np.int64)
    lstm = np.asarray(inputs["lstm_out"], dtype=np.float32)
    return _combine(res.results, lengths, lstm), res


def kernel(lstm_out, lengths, W_w, W_b, v_w):
    (ctx, attn), _ = run(dict(lstm_out=lstm_out, lengths=lengths,
                              W_w=W_w, W_b=W_b, v_w=v_w))
    return ctx, attn


# revision 40
# speedup vs baseline: 1.0102x; 1.0102x over previous
# kernel.py — ConcatAttention on 8 Trainium2 NeuronCores (Bass/Tile, SPMD, no collectives).
#
# reference math (B=4, S=512, H=512, A=128):
#   a[b,i,:] = lstm[b,i] @ W1^T + W_b          (W1 = W_w[:, :H])
#   c[b,j,:] = lstm[b,j] @ W2^T                (W2 = W_w[:, H:])
#   scores[b,i] = sum_j sum_a tanh(a[b,i,a] + c[b,j,a]) * v[a]
#   attn = softmax(where(i < len_b, scores, -1e9), axis=i)
#   context[b] = sum_i attn[b,i] * lstm[b,i]
#
# Algorithm: per (b, a) the function f(t) = sum_j tanh(t + c[b,j,a]) is analytic on
# t in [-2.56, 2.56] (the range a occupies), so a degree-4 Chebyshev interpolant
# (K=5 nodes) reproduces it to ~9.4e-3 end-to-end relative error (tolerance 2e-2):
#   nodes:  F[a,k] = sum_j tanh(t_k + c[a,j])   -> K fused ACT tanh+accum instrs
#   coeffs: coef = F @ Cmat^T                   -> PE transpose + tiny matmul (DCT)
#   eval:   scores[i] = sum_m (v*coef)[a,m] T_m(tau[a,i]) -> K-1 accumulated PE
#           matmuls over the DVE-built Chebyshev basis.
#
# Perf notes (cost-model driven):
#  - All PE matmul inputs are fp16 (1 cycle/row vs fp32's 4); inputs stream in as
#    fp16, halving DMA bytes. End-to-end precision validated at ~9.4e-3.
#  - The i-mask is applied by one extra accumulated matmul adding a -60000 row
#    (fp16-exact, no infs) into the score PSUM; softmax max/merge handles the rest.
#  - Context is produced in partition layout ([h,4] via N=1 matmuls); the softmax
#    row (e | -m | z) leaves early in its own DMA while context computes.
#  - DMA issues are spread over the HWDGE-capable sequencers (SP/ACT); dummy
#    matmuls keep the PE p-state ramped across its idle windows.
#
# Sharding: core = (batch b = core//2, i-half = core%2). Inputs are rotated on the
# host so every core runs the identical program on "its" first 256 rows; the j-sum
# is permutation invariant. Softmax is flash-style per half (m, z, unnormalized e
# and context); halves merge on the host with two scalars per batch.
#
# walrus codegen allows a single sync-wait per TPB instruction, so per engine a
# cheap "gate" op touches each DMA-fed operand first; every real instruction then
# carries at most one unobserved cross-engine producer.

import numpy as np

import concourse.bass as bass
import concourse.mybir as mybir
import concourse.tile as tile
from concourse import bacc
from concourse.bass_utils import run_bass_kernel_spmd
from concourse.tile_rust import add_dep_helper

F32 = mybir.dt.float32
F16 = mybir.dt.float16
AF = mybir.ActivationFunctionType
OP = mybir.AluOpType

B, S, H, A = 4, 512, 512, 128
SH = S // 2          # 256: per-core i-half
K = 5                # Chebyshev nodes (degree 4)
HALF = 2.56          # tau = a / HALF maps a-range into [-1, 1]
N_CORES = 8
NDVE = 0             # leading nodes whose j-sum runs on DVE instead of ACT
NEGF16 = -60000.0    # fp16-exact "minus infinity" for masked queries
WU = 4               # PE p-state warmup matmuls (front)
WK = 46              # PE keep-warm matmuls through the node phase
CONSTS_SP = False    # consts via SP 3rd HWDGE (False: Pool SWDGE)
C_FIRST = True       # all four c-chunks before the a-chunks
GCON_LATE = False    # PE consts gate after projections

# consts layout (one [128, CW] f32 tensor). fp16 blocks are bitcast f32 columns.
C_IDH = 0              # [:, 0:64]    fp16 identity 128x128 (64 f32 cols)
C_TK = 64              # [:, 64:72]   chebyshev node biases (tiled rows)
C_WBH = 72             # [:, 72:73]   W_b / HALF column
C_VW = 73              # [:, 73:74]   v_w column
C_CM = 74              # [:, 74:82]   DCT matrix tiled: rows 32r+k = Cmat^T[k]
C_NM = 82              # [0:1, 82:210] fp16 mask row: 0 active / -60000 masked
C_ONE = 210            # [0:1, 210:211] fp16 [1.0, 0.0] pair
C_ONEF = 211           # [0:1, 211:212] f32 1.0 (transpose identity scalar)
C_CMT = 212            # [:, 212:261] cmt row k tiled on all partitions, per k
CW = 212 + 49
PD = 192               # Pool pre-delay cols before the consts SWDGE issue


def _build_nc():
    nc = bacc.Bacc("TRN2", target_bir_lowering=False, debug=False,
                   num_devices=N_CORES)

    con_d = nc.dram_tensor("consts", [128, CW], F32, kind="ExternalInput")
    # per-h-chunk interleave: cols 0:256 = wts chunk, 256:768 = xt chunk
    ins_d = nc.dram_tensor("ins", [128, 4, 2 * A + S], F16,
                           kind="ExternalInput")

    # outputs: ctx in partition layout; softmax row (e | -m | z) separately
    out_d = nc.dram_tensor("out_all", [128, 4], F32, kind="ExternalOutput")
    outr_d = nc.dram_tensor("out_row", [1, SH + 2], F32, kind="ExternalOutput")

    with tile.TileContext(nc) as tc:
        with (
            tc.tile_pool(name="sb", bufs=1) as sb,
            tc.tile_pool(name="pc", bufs=1, space=bass.MemorySpace.PSUM) as pc,
            tc.tile_pool(name="pa", bufs=1, space=bass.MemorySpace.PSUM) as pa,
            tc.tile_pool(name="px", bufs=1, space=bass.MemorySpace.PSUM) as px,
            tc.tile_pool(name="pt", bufs=1, space=bass.MemorySpace.PSUM) as pt,
            tc.tile_pool(name="pscr", bufs=2,
                         space=bass.MemorySpace.PSUM) as pscr,
        ):
            # --- input DMAs: one per h-chunk so each c-projection starts as
            # soon as its own chunk lands (SP/ACT alternate HWDGE issues) ----
            ins = sb.tile([128, 4, 2 * A + S], F16)
            nc.sync.dma_start(ins[:, 0, :], ins_d.ap()[:, 0, :])
            nc.scalar.dma_start(ins[:, 1, :], ins_d.ap()[:, 1, :])
            nc.sync.dma_start(ins[:, 2, :], ins_d.ap()[:, 2, :])
            nc.scalar.dma_start(ins[:, 3, :], ins_d.ap()[:, 3, :])

            def wts(hc):
                return ins[:, hc, 0:2 * A]

            def xts(hc, lo, hi):
                return ins[:, hc, 2 * A + lo:2 * A + hi]

            con = sb.tile([128, CW], F32)
            if CONSTS_SP:
                nc.sync.dma_start(con[:, :], con_d.ap())
            else:
                # dummy Pool op first: delays the SWDGE generation just enough
                # that the xt transfers win the shared DMA queue
                pdum = sb.tile([128, PD // 4], F32)
                nc.gpsimd.memset(pdum[:, :], 0.0)
                nc.gpsimd.dma_start(con[:, :], con_d.ap())

            identh = con[:, C_IDH:C_IDH + 64].bitcast(F16)   # [128,128] fp16
            tks = con[:, C_TK:C_TK + K]
            wbh = con[:, C_WBH:C_WBH + 1]
            vw = con[:, C_VW:C_VW + 1]
            cmt4 = con[:, C_CM:C_CM + K]                     # tiled per 32-blk
            nmk = con[0:1, C_NM:C_NM + 128].bitcast(F16)     # [1,256] fp16
            onec = con[0:1, C_ONE:C_ONE + 1].bitcast(F16)    # [1,2] fp16
            onef = con[0:1, C_ONEF:C_ONEF + 1]               # [1,1] f32 1.0
            cmtt = con[:, C_CMT:C_CMT + K * K]                # row-tiled cmt

            # --- engine gates: pre-observe each DMA per engine --------------
            g_wts = nc.tensor.ldweights(ins[:, 0, 0:1])
            if not GCON_LATE:
                g_con = nc.tensor.ldweights(identh[:, 0:1])
                add_dep_helper(g_con.ins, g_wts.ins, False, "gate order")
            dummy_a = sb.tile([A, 1], F32)
            # also preloads the tanh/exp ACT table while DMAs stream
            g_act = nc.scalar.activation(dummy_a[:, :], tks[:, 0:1], AF.Tanh,
                                         bias=tks[:, 0:1])
            dummy_d = sb.tile([1, 1], F32)
            g_dve = nc.vector.tensor_copy(dummy_d[0:1, 0:1], con[0:1, 0:1])

            # --- PE p-state warmup on wts while xt streams ------------------
            # (shares the ftp-tag PSUM bank; warmup is long dead before the
            # DCT transpose reuses it)
            wu_ps = pt.tile([32, 128], F32, tag="ftp")
            for i in range(WU):
                nc.tensor.matmul(wu_ps[0:1, 0:128], ins[:, 0, 0:1],
                                  ins[:, 0, 0:128], start=True, stop=True)

            # --- projections on PE: c first (feeds nodes), then a -----------
            c_ps = pc.tile([A, S], F32)
            a_ps = pa.tile([A, SH], F32, tag="a_ps")
            if C_FIRST:
                order = [("c", 0), ("c", 1), ("c", 2), ("c", 3),
                         ("a", 0), ("a", 1), ("a", 2), ("a", 3)]
            else:
                order = [("c", 0), ("c", 1), ("a", 0), ("a", 1),
                         ("c", 2), ("c", 3), ("a", 2), ("a", 3)]
            for kind, hc in order:
                if kind == "c":
                    nc.tensor.matmul(c_ps[:, :], wts(hc)[:, A:2 * A],
                                     xts(hc, 0, S), start=(hc == 0),
                                     stop=(hc == 3), skip_group_check=True)
                else:
                    nc.tensor.matmul(a_ps[:, :], wts(hc)[:, 0:A],
                                     xts(hc, 0, SH), start=(hc == 0),
                                     stop=(hc == 3), skip_group_check=True)
            if GCON_LATE:
                # PE observes the consts DMA before its first use (transposes)
                g_con = nc.tensor.ldweights(identh[:, 0:1])

            # --- rebuild x[i,h] (fp16) for the context matmul ---------------
            xh0 = sb.tile([128, H], F16)
            xh1 = sb.tile([128, H], F16)
            xh_sb = [xh0, xh1]
            for sc in range(2):
                xps = px.tile([128, 4, 128], F16, tag="xps")
                for hc in range(4):
                    nc.tensor.transpose(xps[:, hc, :],
                                        xts(hc, sc * 128, (sc + 1) * 128),
                                        identh)
                nc.vector.tensor_copy(xh_sb[sc][:, :], xps[:, :, :])

            # --- tau (=T_1) and basis recurrence, all on DVE ----------------
            basis = sb.tile([A, K, SH], F16)  # slots m=1..K-1 used
            nc.vector.tensor_scalar(basis[:, 1, :], a_ps[:, :], 1.0 / HALF,
                                    wbh, OP.mult, OP.add)

            # --- Chebyshev node sums on ACT (tanh + fused row-sum) ----------
            # fnode is padded to 32 cols for the DVE block-transpose; the pad
            # is zeroed on ACT (no cross-engine dep) right after the gate
            fnode = sb.tile([A, 8], F32)
            for k in range(K):
                scr = pscr.tile([A, S], F32, tag="scr")
                nc.scalar.activation(scr[:, :], c_ps[:, :], AF.Tanh,
                                     bias=tks[:, k:k + 1],
                                     accum_out=fnode[:, k:k + 1])

            # --- basis: T_2k = 2*T_k^2-1, T_2k+1 = 2*T_k*T_k+1 - T_1 --------
            um = sb.tile([A, SH], F16)
            for m in range(2, K):
                hm = m // 2 if m % 2 == 0 else (m - 1) // 2
                nc.vector.tensor_mul(um[:, :], basis[:, hm, :],
                                     basis[:, hm + (m % 2), :])
                if m % 2 == 0:
                    nc.vector.tensor_scalar(basis[:, m, :], um[:, :], 2.0,
                                            -1.0, OP.mult, OP.add)
                else:
                    nc.vector.scalar_tensor_tensor(basis[:, m, :], um[:, :],
                                                   2.0, basis[:, 1, :],
                                                   OP.mult, OP.subtract)

            # --- keep PE p-state ramped through the ACT node phase ----------
            for i in range(WK):
                nc.tensor.matmul(wu_ps[0:1, 0:128], ins[:, 0, 0:1],
                                 ins[:, 0, 0:128], start=True, stop=True)

            # --- node values -> v * Chebyshev coefficients ------------------
            # incremental DCT on DVE: after node k lands, rank-1 update
            # coef += F[:,k] (x) cmt[k,:] via a per-partition-scalar op, so
            # only the last tiny update sits after the final node
            tailp = pt.tile([128, 16], F32, tag="tail")
            # v_w is folded into cmtt on the host, and coefa accumulates in
            # fp16, so the score matmuls read it directly (no convert hop)
            coefa = sb.tile([A, 8], F16)
            for k in range(K):
                blk = cmtt[:, k * K:(k + 1) * K]
                if k == 0:
                    nc.vector.tensor_scalar(coefa[:, 0:K], blk,
                                            fnode[:, 0:1], None, OP.mult)
                else:
                    nc.vector.scalar_tensor_tensor(coefa[:, 0:K], blk,
                                                   fnode[:, k:k + 1],
                                                   coefa[:, 0:K],
                                                   OP.mult, OP.add)


            # --- scores via K-1 accumulated matmuls + mask row --------------
            sco = pa.tile([1, SH], F32, tag="sco")
            for m in range(1, K):
                nc.tensor.matmul(sco[:, :], coefa[:, m:m + 1], basis[:, m, :],
                                 start=(m == 1), stop=False,
                                 skip_group_check=True)
            nc.tensor.matmul(sco[:, :], onec[:, 0:1], nmk,
                             start=False, stop=True, skip_group_check=True)

            # --- flash softmax half: negm, e, z -----------------------------
            negm = sb.tile([1, 1], F32)
            nc.vector.tensor_reduce(negm[:, :], sco[:, :],
                                    axis=mybir.AxisListType.X, op=OP.max,
                                    negate=True)
            # ACT observes sco while DVE reduces, so exp carries only negm
            dummy_s = sb.tile([1, 1], F32)
            g_sco = nc.scalar.activation(dummy_s[:, :], sco[0:1, 0:1],
                                         AF.Identity)
            # exp writes straight into the row-packed output tile
            packr = sb.tile([1, SH + 2], F32)
            e_sb = packr[0:1, 0:SH]
            eop = nc.scalar.activation(e_sb, sco[:, :], AF.Exp,
                                       bias=negm[0:1, 0:1])
            add_dep_helper(eop.ins, g_sco.ins, False, "gate order")
            # z on DVE (parallel with the PE e-transposes); negm via ACT after
            zop = nc.vector.tensor_reduce(packr[0:1, SH + 1:SH + 2], e_sb,
                                          axis=mybir.AxisListType.X, op=OP.add)
            pmop = nc.scalar.activation(packr[0:1, SH:SH + 1], negm[:, :],
                                        AF.Identity)
            add_dep_helper(pmop.ins, eop.ins, False, "pack order")
            add_dep_helper(pmop.ins, zop.ins, False, "row pack complete")
            nc.scalar.dma_start(outr_d.ap(), packr[:, :])

            # --- context in partition layout: cux[h,hc] ---------------------
            etp = tailp[:, 8:10]
            for ch in range(2):
                nc.tensor.transpose(etp[:, ch:ch + 1],
                                    e_sb[0:1, ch * 128:(ch + 1) * 128],
                                    onef)
            et = sb.tile([128, 2], F16)
            nc.vector.tensor_copy(et[:, :], etp[:, :])
            cux = tailp[:, 10:14]
            for hc in range(4):
                for sc in range(2):
                    nc.tensor.matmul(cux[:, hc:hc + 1],
                                     xh_sb[sc][:, hc * 128:(hc + 1) * 128],
                                     et[:, sc:sc + 1],
                                     start=(sc == 0), stop=(sc == 1),
                                     skip_group_check=True)

            # --- ctx leaves in a second small DMA ---------------------------
            pack = sb.tile([128, 4], F32)
            nc.vector.tensor_copy(pack[:, :], cux)
            nc.sync.dma_start(out_d.ap(), pack[:, :])

    nc.compile()
    return nc


_NC_CACHE = None


def _get_nc():
    global _NC_CACHE
    if _NC_CACHE is None:
        _NC_CACHE = _build_nc()
    return _NC_CACHE


def _host_inputs(lstm_out, lengths, W_w, W_b, v_w):
    lstm = np.ascontiguousarray(np.asarray(lstm_out), dtype=np.float32)
    W_w = np.asarray(W_w, dtype=np.float32)
    W_b = np.asarray(W_b, dtype=np.float32)
    v_w = np.asarray(v_w, dtype=np.float32)
    lengths = np.asarray(lengths).astype(np.int64)

    wts = np.empty((H, 2 * A), np.float16)
    wts[:, 0:A] = W_w[:, :H].T          # W1^T
    wts[:, A:2 * A] = W_w[:, H:].T      # W2^T
    wts = wts.reshape(4, 128, 2 * A).transpose(1, 0, 2)  # [128, 4hc, 2A]

    kk = np.arange(K)
    tk = (HALF * np.cos((2 * kk + 1) * np.pi / (2 * K))).astype(np.float32)
    mm = np.arange(K)
    cmat = np.cos(np.outer(mm, (2 * kk + 1)) * np.pi / (2 * K)) * (2.0 / K)
    cmat[0] *= 0.5

    con_base = np.zeros((128, CW), np.float32)
    con_base[:, C_IDH:C_IDH + 64] = np.eye(128, dtype=np.float16).view(np.float32)
    con_base[:, C_TK:C_TK + K] = np.tile(tk[None, :], (128, 1))
    con_base[:, C_WBH:C_WBH + 1] = (W_b / np.float32(HALF))[:, None]
    con_base[:, C_VW:C_VW + 1] = v_w[:, None]
    for r in range(4):
        con_base[32 * r:32 * r + K, C_CM:C_CM + K] = cmat.T.astype(np.float32)
    for k in range(K):
        con_base[:, C_CMT + k * K:C_CMT + (k + 1) * K] = \
            v_w[:, None] * cmat[:, k].astype(np.float32)[None, :]
    con_base[0:1, C_ONE:C_ONE + 1] = (
        np.array([[1.0, 0.0]], np.float16).view(np.float32))
    con_base[0, C_ONEF] = 1.0

    mask01 = (np.arange(S)[None, :] < lengths[:, None])

    in_maps = []
    for core in range(N_CORES):
        b, half = core // 2, core % 2
        rot = half * SH
        x_rot = np.concatenate([lstm[b, rot:], lstm[b, :rot]], axis=0)
        x16 = x_rot.astype(np.float16)
        xt = x16.T.reshape(4, 128, S).transpose(1, 0, 2)   # [128, 4, S]
        ins = np.empty((128, 4, 2 * A + S), np.float16)
        ins[:, :, 0:2 * A] = wts
        ins[:, :, 2 * A:] = xt
        m01 = mask01[b, rot:rot + SH]
        con = con_base.copy()
        nmk16 = np.where(m01, np.float16(0.0),
                         np.float16(NEGF16)).astype(np.float16)
        con[0:1, C_NM:C_NM + 128] = nmk16.reshape(1, SH).view(np.float32)
        in_maps.append({
            "consts": con,
            "ins": ins,
        })
    return in_maps


def _combine(results, lengths, lstm):
    attn = np.zeros((B, S), np.float32)
    ctx = np.zeros((B, H), np.float32)
    for b in range(B):
        if lengths[b] == 0:
            attn[b] = 1.0 / S
            ctx[b] = lstm[b].mean(axis=0)
            continue
        mzs, es, cxs = [], [], []
        for h in range(2):
            o = results[2 * b + h]["out_all"].astype(np.float64)
            r = results[2 * b + h]["out_row"][0].astype(np.float64)
            m_h = -r[SH]
            z_h = r[SH + 1]
            e_h = r[0:SH]
            c_h = o[:, 0:4].T.reshape(H)
            mzs.append((m_h, z_h)); es.append(e_h); cxs.append(c_h)
        mg = max(m for m, _ in mzs)
        acc_z = 0.0
        aa = []
        for (m_h, z_h) in mzs:
            a_h = np.exp(m_h - mg) if np.isfinite(m_h) else 0.0
            aa.append(a_h)
            acc_z += a_h * z_h
        attn[b, :SH] = aa[0] * es[0] / acc_z
        attn[b, SH:] = aa[1] * es[1] / acc_z
        ctx[b] = (aa[0] * cxs[0] + aa[1] * cxs[1]) / acc_z
    return ctx, attn


def run(inputs, trace=False):
    """Internal entry that also exposes tracing; returns ((ctx, attn), results)."""
    nc = _get_nc()
    in_maps = _host_inputs(**inputs)
    res = run_bass_kernel_spmd(nc, in_maps, core_ids=list(range(N_CORES)),
                               trace=trace)
    lengths = np.asarray(inputs["lengths"]).astype(np.int64)
    lstm = np.asarray(inputs["lstm_out"], dtype=np.float32)
    return _combine(res.results, lengths, lstm), res


def kernel(lstm_out, lengths, W_w, W_b, v_w):
    (ctx, attn), _ = run(dict(lstm_out=lstm_out, lengths=lengths,
                              W_w=W_w, W_b=W_b, v_w=v_w))
    return ctx, attn
